# revision 29
# baseline (speedup 1.0000x reference)
"""NemoGPT (L=3, H=4, D=16, E=64, V=32000, B=64, T=64) on 8 Trainium2 cores.

Strategy: data-parallel over batch (8 batches/core = 512 tokens). Each core
runs the full transformer on its shard and streams its [512, 32000] f32
logits to DRAM. No collectives; the host concatenates per-core outputs.

Device structure (per core): 4 "pair" chunks of 128 tokens (= 2 batches),
processed in two groups of 2 pairs. Within a group the transformer runs
layer-major (minimizes ACT table-set switches between exp/ln and gelu);
each group's lm_head streaming overlaps the other group's compute.

Key layout tricks:
  - residual x token-major [128, E]; LN stats along the free dim; LN affine
    folded into the consuming weights (bias via a ones-row on the K dim)
  - scoresT [s, t] per head so exp(scoresT) is directly the attn lhsT
  - softmax without max-subtraction (scores ~ +-0.1); normalizer from a
    ones-column appended to V; all matmuls at PE row-group 0
  - MLP hidden computed transposed (uT = W1.T @ xhatT) so mlp2's lhsT is
    just gelu(uT) -- no transposes in the MLP
  - weight-side matmuls in bf16 (f32 PSUM accumulate); scores/attn in f32
  - lm_head: bf16 [65,128]x[65,500] -> f32 PSUM -> SBUF stage [128, 4000]
    -> 2 MiB DMAs
"""

import sys

for _p in ("/opt/trn_rl_repo", "/root/.axon_site", "/root/.axon_site/_ro/pypackages"):
    if _p not in sys.path:
        sys.path.insert(0, _p)

import numpy as np

L, H, D, E, V = 3, 4, 16, 64, 32000
B, T = 64, 64
NCORES = 8
BL = B // NCORES            # batches per core
N = BL * T                  # tokens per core
P = 128                     # tokens per pair-chunk (2 batches)
NPAIR = N // P
SCALE = 1.0 / np.sqrt(E)
EPS = 1e-5
VC = 500                    # vocab cols per matmul
VG = 8                      # matmul chunks per staged DMA
NV = V // VC
NEG = -1.0e30
GROUPS = ((0,), (1,), (2,), (3,))

_PROG = None


def _build_program():
    import concourse.bass as bass
    import concourse.tile as tile
    from concourse import bacc, mybir
    from contextlib import ExitStack

    f32 = mybir.dt.float32
    bf16 = mybir.dt.bfloat16
    i32 = mybir.dt.int32
    A = mybir.ActivationFunctionType
    Op = mybir.AluOpType

    nc = bacc.Bacc("TRN2", target_bir_lowering=False, debug=False,
                   num_devices=NCORES)

    # ---- DRAM parameters ----
    d_idx = nc.dram_tensor("idx", [N], i32, kind="ExternalInput").ap()
    d_temb = nc.dram_tensor("temb", [V, E], f32, kind="ExternalInput").ap()
    d_wq = nc.dram_tensor("wq", [L, E + 1, E], bf16, kind="ExternalInput").ap()
    d_wk = nc.dram_tensor("wk", [L, E + 1, E], bf16, kind="ExternalInput").ap()
    d_wv = nc.dram_tensor("wv", [L, E + 1, 68], bf16, kind="ExternalInput").ap()
    d_wp = nc.dram_tensor("wp", [L, E + 1, E], bf16, kind="ExternalInput").ap()
    d_w1 = nc.dram_tensor("w1", [L, E + 1, 256], bf16, kind="ExternalInput").ap()
    d_w2 = nc.dram_tensor("w2", [L, 256, E], bf16, kind="ExternalInput").ap()
    d_b2 = nc.dram_tensor("b2", [L, 1, E], f32, kind="ExternalInput").ap()
    d_wlm = nc.dram_tensor("wlm", [E + 1, V], bf16, kind="ExternalInput").ap()
    d_pos = nc.dram_tensor("pos2", [P, E], f32, kind="ExternalInput").ap()
    d_mask = nc.dram_tensor("maskb", [P, 4 * T], f32, kind="ExternalInput").ap()
    d_ident = nc.dram_tensor("ident", [P, P], f32, kind="ExternalInput").ap()
    d_out = nc.dram_tensor("out", [N, V], f32, kind="ExternalOutput").ap()

    with tile.TileContext(nc) as tc:
        with ExitStack() as ctx:
            consts = ctx.enter_context(tc.tile_pool(name="consts", bufs=1))
            acts = ctx.enter_context(tc.tile_pool(name="acts", bufs=3))
            tmp = ctx.enter_context(tc.tile_pool(name="tmp", bufs=3))
            sbt = ctx.enter_context(tc.tile_pool(name="sbt", bufs=3))
            stg = ctx.enter_context(tc.tile_pool(name="stg", bufs=3))
            ps = ctx.enter_context(tc.tile_pool(name="ps", bufs=3, space="PSUM"))
            psu = ctx.enter_context(tc.tile_pool(name="psu", bufs=2, space="PSUM"))
            pslg = ctx.enter_context(tc.tile_pool(name="pslg", bufs=3, space="PSUM"))

            # ---- constants into SBUF ----
            wq_sb, wk_sb, wv_sb, wp_sb, w1_sb = [], [], [], [], []
            w2a_sb, w2b_sb, b2_sb = [], [], []
            for l in range(L):
                tq = consts.tile([E + 1, E], bf16, tag=f"wq{l}")
                nc.sync.dma_start(tq[:], d_wq[l])
                wq_sb.append(tq)
                tk = consts.tile([E + 1, E], bf16, tag=f"wk{l}")
                nc.sync.dma_start(tk[:], d_wk[l])
                wk_sb.append(tk)
                tv = consts.tile([E + 1, 68], bf16, tag=f"wv{l}")
                nc.sync.dma_start(tv[:], d_wv[l])
                wv_sb.append(tv)
                tp = consts.tile([E + 1, E], bf16, tag=f"wp{l}")
                nc.sync.dma_start(tp[:], d_wp[l])
                wp_sb.append(tp)
                t1 = consts.tile([E + 1, 256], bf16, tag=f"w1{l}")
                nc.sync.dma_start(t1[:], d_w1[l])
                w1_sb.append(t1)
                t2a = consts.tile([128, E], bf16, tag=f"w2a{l}")
                nc.sync.dma_start(t2a[:], d_w2[l, 0:128])
                w2a_sb.append(t2a)
                t2b = consts.tile([128, E], bf16, tag=f"w2b{l}")
                nc.sync.dma_start(t2b[:], d_w2[l, 128:256])
                w2b_sb.append(t2b)
                tb2 = consts.tile([P, E], f32, tag=f"b2{l}")
                b2bc = bass.AP(tensor=d_b2.tensor, offset=d_b2[l, 0].offset,
                               ap=[[0, P]] + [list(a) for a in d_b2[l, 0].ap])
                nc.gpsimd.dma_start(tb2[:], b2bc)
                b2_sb.append(tb2)
            wlm_sb = consts.tile([E + 1, V], bf16)
            nc.sync.dma_start(wlm_sb[:], d_wlm[:])
            pos_sb = consts.tile([P, E], f32)
            nc.sync.dma_start(pos_sb[:], d_pos[:])
            mask_sb = consts.tile([P, 4 * T], f32)
            nc.sync.dma_start(mask_sb[:], d_mask[:])
            ident_sb = consts.tile([P, P], f32)
            nc.sync.dma_start(ident_sb[:], d_ident[:])
            eps_sb = consts.tile([P, 1], f32)
            nc.vector.memset(eps_sb[:], EPS)

            from concourse.tile import add_dep_helper

            def layernorm(x, name, marks=None):
                """token-major LN: xhat [P, E] (affine folded into weights)."""
                st6 = tmp.tile([P, 6], f32, tag=f"st6_{name}")
                nc.vector.bn_stats(st6[:], x[:])
                mv = tmp.tile([P, 2], f32, tag=f"mv_{name}")
                nc.vector.bn_aggr(mv[:], st6[:])
                lnv = tmp.tile([P, 1], f32, tag=f"lnv_{name}")
                i_ln = nc.scalar.activation(lnv[:], mv[:, 1:2], A.Ln,
                                            bias=eps_sb[:])
                rstd = tmp.tile([P, 1], f32, tag=f"rstd_{name}")
                i_ex = nc.scalar.activation(rstd[:], lnv[:], A.Exp, scale=-0.5)
                xh = tmp.tile([P, E], f32, tag=f"xh_{name}")
                nc.vector.scalar_tensor_tensor(
                    out=xh[:], in0=x[:], scalar=mv[:, 0:1],
                    in1=rstd[:].to_broadcast([P, E]),
                    op0=Op.subtract, op1=Op.mult)
                if marks is not None:
                    marks.append((i_ln, i_ex))
                return xh

            def transpose_aug(src, name, copy_eng, dtype=f32):
                """[P, E]-tile -> [E+1, P] transposed + trailing ones row."""
                tps = ps.tile([E, P], f32, tag="ps")
                nc.tensor.transpose(tps[:], src[:], ident_sb[:])
                out = sbt.tile([E + 1, P], dtype, tag=f"tr_{name}")
                copy_eng(out[0:E, :], tps[:])
                nc.vector.memset(out[E:E + 1, :], 1.0)
                return out

            import concourse.bass as bass2

            def embed(p):
                idx_sb = tmp.tile([P, 1], i32, tag="idx")
                nc.sync.dma_start(idx_sb[:], d_idx[p * P:(p + 1) * P, None])
                xg = tmp.tile([P, E], f32, tag="xg")
                nc.gpsimd.indirect_dma_start(
                    out=xg[:], out_offset=None, in_=d_temb[:],
                    in_offset=bass2.IndirectOffsetOnAxis(ap=idx_sb[:, :1],
                                                         axis=0))
                x = acts.tile([P, E], f32, tag="x0")
                nc.vector.tensor_add(x[:], xg[:], pos_sb[:])
                return x

            def attention(x, l, entry_marks):
                xh = layernorm(x, "ln1", marks=entry_marks)
                hT = transpose_aug(xh, "h", nc.vector.tensor_copy, dtype=bf16)
                qps = ps.tile([D, H * P], f32, tag="ps")
                kps = ps.tile([D, H * P], f32, tag="ps")
                for h in range(H):
                    nc.tensor.matmul(qps[0:D, P * h:P * (h + 1)],
                                     lhsT=wq_sb[l][:, D * h:D * (h + 1)],
                                     rhs=hT[:])
                    nc.tensor.matmul(kps[0:D, P * h:P * (h + 1)],
                                     lhsT=wk_sb[l][:, D * h:D * (h + 1)],
                                     rhs=hT[:])
                qT = sbt.tile([D, H * P], f32, tag="qT")
                nc.scalar.copy(qT[:], qps[:])
                kT = sbt.tile([D, H * P], f32, tag="kT")
                nc.vector.tensor_copy(kT[:], kps[:])
                vps = ps.tile([P, 68], f32, tag="ps")
                nc.tensor.matmul(vps[:], lhsT=hT[:], rhs=wv_sb[l][:])
                v = sbt.tile([P, 68], f32, tag="v")
                nc.scalar.copy(v[:], vps[:])

                scps = ps.tile([P, 4 * T], f32, tag="ps")
                for h in range(H):
                    for b in range(2):
                        r0 = 64 * b
                        nc.tensor.matmul(
                            scps[r0:r0 + 64, 64 * h:64 * h + 64],
                            lhsT=kT[0:D, P * h + r0:P * h + r0 + 64],
                            rhs=qT[0:D, P * h + r0:P * h + r0 + 64])
                sm = sbt.tile([P, 4 * T], f32, tag="sm")
                nc.vector.scalar_tensor_tensor(
                    out=sm[:], in0=scps[:], scalar=float(SCALE),
                    in1=mask_sb[:], op0=Op.mult, op1=Op.add)
                ex = sbt.tile([P, 4 * T], f32, tag="ex")
                nc.scalar.activation(ex[:], sm[:], A.Exp)

                atps = ps.tile([P, 68], f32, tag="ps")
                for h in range(H):
                    for b in range(2):
                        r0 = 64 * b
                        nc.tensor.matmul(
                            atps[r0:r0 + 64, 17 * h:17 * h + 17],
                            lhsT=ex[r0:r0 + 64, 64 * h:64 * h + 64],
                            rhs=v[r0:r0 + 64, 17 * h:17 * h + 17])
                rr = tmp.tile([P, H], f32, tag="rr")
                nc.vector.reciprocal(rr[:], atps[:, 16::17])
                attn = tmp.tile([P, E], f32, tag="attn")
                at3 = atps[:].rearrange("p (h c) -> p h c", c=17)[:, :, 0:D]
                nc.vector.tensor_tensor(
                    out=attn[:].rearrange("p (h d) -> p h d", d=D),
                    in0=at3, in1=rr[:].to_broadcast([P, H, D]), op=Op.mult)

                aT = transpose_aug(attn, "a", nc.scalar.copy, dtype=bf16)
                pjps = ps.tile([P, E], f32, tag="ps")
                nc.tensor.matmul(pjps[:], lhsT=aT[:], rhs=wp_sb[l][:])
                x2 = acts.tile([P, E], f32, tag="x2")
                nc.vector.tensor_add(x2[:], x[:], pjps[:])
                return x2

            def mlp(x2, l, lastnl_marks, gelu_insts):
                xh2 = layernorm(x2, "ln2", marks=lastnl_marks)
                h2T = transpose_aug(xh2, "h2", nc.vector.tensor_copy,
                                    dtype=bf16)
                u1ps = psu.tile([P, P], f32, tag="psu")
                nc.tensor.matmul(u1ps[:], lhsT=w1_sb[l][:, 0:128], rhs=h2T[:])
                g1T = sbt.tile([P, P], bf16, tag="g1T")
                gelu_insts.append(nc.scalar.activation(g1T[:], u1ps[:], A.Gelu))
                u2ps = psu.tile([P, P], f32, tag="psu")
                nc.tensor.matmul(u2ps[:], lhsT=w1_sb[l][:, 128:256], rhs=h2T[:])
                g2T = sbt.tile([P, P], bf16, tag="g2T")
                gelu_insts.append(nc.scalar.activation(g2T[:], u2ps[:], A.Gelu))
                dps = ps.tile([P, E], f32, tag="ps")
                nc.tensor.matmul(dps[:], lhsT=g1T[:], rhs=w2a_sb[l][:],
                                 start=True, stop=False)
                nc.tensor.matmul(dps[:], lhsT=g2T[:], rhs=w2b_sb[l][:],
                                 start=False, stop=True)
                x3 = acts.tile([P, E], f32, tag="x3")
                nc.vector.tensor_add(x3[:], dps[:], b2_sb[l][:])
                x4 = acts.tile([P, E], f32, tag="x4")
                nc.vector.tensor_add(x4[:], x3[:], x2[:])
                return x4

            def lm_head(x, p, entry_marks):
                xf = layernorm(x, "lnf", marks=entry_marks)
                fT = transpose_aug(xf, "f", nc.vector.tensor_copy, dtype=bf16)
                for c2 in range(NV // VG):
                    stage = stg.tile([P, VG * VC], f32, tag="stage")
                    for j in range(VG):
                        c = VG * c2 + j
                        lg = pslg.tile([P, VC], f32, tag="lg")
                        nc.tensor.matmul(lg[:], lhsT=fT[:],
                                         rhs=wlm_sb[:, VC * c:VC * (c + 1)])
                        eng = (nc.vector.tensor_copy if j % 8 < 5
                               else nc.scalar.copy)
                        eng(stage[:, VC * j:VC * (j + 1)], lg[:])
                    nc.sync.dma_start(
                        d_out[p * P:(p + 1) * P,
                              VG * VC * c2:VG * VC * (c2 + 1)],
                        stage[:])

            # ACT table-set ordering: alternate clean "eras" of the
            # natural_log_exp set (LN/softmax ops) and the gelu set. Every
            # gelu cluster waits (ordering-only) on all pending nl-set exp
            # ops; every later Ln waits on the previous gelu cluster. This
            # keeps walrus's PSEUDO_LOAD_ACT_FUNC_SET count at ~2 per layer.
            prev_gelus = []
            pending_nl = []
            for grp in GROUPS:
                xs = {p: embed(p) for p in grp}
                for l in range(L):
                    entry, lastnl, gelus = [], [], []
                    for p in grp:
                        xs[p] = attention(xs[p], l, entry)
                    for p in grp:
                        xs[p] = mlp(xs[p], l, lastnl, gelus)
                    for i_ln, _ in entry:
                        for g in prev_gelus:
                            add_dep_helper(i_ln.ins, g.ins, sync=False,
                                           reason="act-set order")
                    for g in gelus:
                        for _, i_ex in pending_nl + lastnl:
                            add_dep_helper(g.ins, i_ex.ins, sync=False,
                                           reason="act-set order")
                    prev_gelus = gelus
                    pending_nl = []
                lnf_entry = []
                for p in grp:
                    lm_head(xs[p], p, lnf_entry)
                for i_ln, _ in lnf_entry:
                    for g in prev_gelus:
                        add_dep_helper(i_ln.ins, g.ins, sync=False,
                                       reason="act-set order")
                pending_nl = list(lnf_entry)

    nc.compile()
    return nc


def _prep_inputs(idx, tok_emb, pos_emb, Wq, Wk, Wv, Wproj, bproj,
                 ln1_g, ln1_b, ln2_g, ln2_b, W1, b1, W2, b2,
                 lnf_g, lnf_b, Wlm, blm):
    """Host-side weight folding/packing. Returns (shared inputs, per-core idx)."""
    import ml_dtypes
    f = np.float32
    bf = ml_dtypes.bfloat16
    idx = np.asarray(idx).astype(np.int32)
    tok_emb = np.asarray(tok_emb, f)
    pos_emb = np.asarray(pos_emb, f)
    Wq, Wk, Wv = np.asarray(Wq, f), np.asarray(Wk, f), np.asarray(Wv, f)
    Wproj, bproj = np.asarray(Wproj, f), np.asarray(bproj, f)
    ln1_g, ln1_b = np.asarray(ln1_g, f), np.asarray(ln1_b, f)
    ln2_g, ln2_b = np.asarray(ln2_g, f), np.asarray(ln2_b, f)
    W1, b1 = np.asarray(W1, f), np.asarray(b1, f)
    W2, b2 = np.asarray(W2, f), np.asarray(b2, f)
    lnf_g, lnf_b = np.asarray(lnf_g, f), np.asarray(lnf_b, f)
    Wlm, blm = np.asarray(Wlm, f), np.asarray(blm, f)

    wq_p = np.zeros((L, E + 1, E), f)
    wk_p = np.zeros((L, E + 1, E), f)
    wv_p = np.zeros((L, E + 1, 68), f)
    wp_p = np.zeros((L, E + 1, E), f)
    w1_p = np.zeros((L, E + 1, 256), f)
    w2_p = np.zeros((L, 256, E), f)
    b2_p = np.zeros((L, 1, E), f)
    for l in range(L):
        g1, b1l = ln1_g[l][:, None], ln1_b[l]
        for h in range(H):
            wq_p[l, :E, D * h:D * h + D] = g1 * Wq[l, h]
            wq_p[l, E, D * h:D * h + D] = b1l @ Wq[l, h]
            wk_p[l, :E, D * h:D * h + D] = g1 * Wk[l, h]
            wk_p[l, E, D * h:D * h + D] = b1l @ Wk[l, h]
            wv_p[l, :E, 17 * h:17 * h + D] = g1 * Wv[l, h]
            wv_p[l, E, 17 * h:17 * h + D] = b1l @ Wv[l, h]
            wv_p[l, E, 17 * h + D] = 1.0          # ones-column -> row sums
        wp_p[l, :E] = Wproj[l]
        wp_p[l, E] = bproj[l]
        w1_p[l, :E] = ln2_g[l][:, None] * W1[l]
        w1_p[l, E] = ln2_b[l] @ W1[l] + b1[l]
        w2_p[l] = W2[l]
        b2_p[l, 0] = b2[l]
    wlm_p = np.empty((E + 1, V), f)
    wlm_p[:E] = lnf_g[:, None] * Wlm
    wlm_p[E] = lnf_b @ Wlm + blm

    pos2 = np.concatenate([pos_emb, pos_emb], 0)          # [128, 64]
    m = np.where(np.arange(T)[:, None] <= np.arange(T)[None, :], 0, NEG)
    maskb = np.tile(np.concatenate([m, m], 0), (1, H)).astype(f)  # [128, 256]
    ident = np.eye(P, dtype=f)

    shared = dict(temb=tok_emb, wq=wq_p.astype(bf), wk=wk_p.astype(bf),
                  wv=wv_p.astype(bf), wp=wp_p.astype(bf), w1=w1_p.astype(bf),
                  w2=w2_p.astype(bf), b2=b2_p, wlm=wlm_p.astype(bf),
                  pos2=pos2, maskb=maskb, ident=ident)
    idx_cores = [idx[BL * i:BL * (i + 1)].reshape(N) for i in range(NCORES)]
    return shared, idx_cores


def make_in_maps(**inputs):
    shared, idx_cores = _prep_inputs(**inputs)
    return [dict(shared, idx=idx_cores[i]) for i in range(NCORES)]


def get_program():
    global _PROG
    if _PROG is None:
        _PROG = _build_program()
    return _PROG


def kernel(**inputs):
    from concourse.bass_utils import run_bass_kernel_spmd

    nc = get_program()
    in_maps = make_in_maps(**inputs)
    res = run_bass_kernel_spmd(nc, in_maps, list(range(NCORES)))
    outs = [res.results[i]["out"].reshape(BL, T, V) for i in range(NCORES)]
    return np.concatenate(outs, 0)


# revision 30
# speedup vs baseline: 1.0599x; 1.0599x over previous
"""NemoGPT (L=3, H=4, D=16, E=64, V=32000, B=64, T=64) on 8 Trainium2 cores.

Strategy: data-parallel over batch (8 batches/core = 512 tokens). Each core
runs the full transformer on its shard and streams its [512, 32000] f32
logits to DRAM. No collectives; the host concatenates per-core outputs.

Device structure (per core): 4 "pair" chunks of 128 tokens (= 2 batches),
processed in two groups of 2 pairs. Within a group the transformer runs
layer-major (minimizes ACT table-set switches between exp/ln and gelu);
each group's lm_head streaming overlaps the other group's compute.

Key layout tricks:
  - residual x token-major [128, E]; LN stats along the free dim; LN affine
    folded into the consuming weights (bias via a ones-row on the K dim)
  - scoresT [s, t] per head so exp(scoresT) is directly the attn lhsT
  - softmax without max-subtraction (scores ~ +-0.1); normalizer from a
    ones-column appended to V; all matmuls at PE row-group 0
  - MLP hidden computed transposed (uT = W1.T @ xhatT) so mlp2's lhsT is
    just gelu(uT) -- no transposes in the MLP
  - weight-side matmuls in bf16 (f32 PSUM accumulate); scores/attn in f32
  - lm_head: bf16 [65,128]x[65,500] -> f32 PSUM -> SBUF stage [128, 4000]
    -> 2 MiB DMAs
"""

import sys

for _p in ("/opt/trn_rl_repo", "/root/.axon_site", "/root/.axon_site/_ro/pypackages"):
    if _p not in sys.path:
        sys.path.insert(0, _p)

import numpy as np

L, H, D, E, V = 3, 4, 16, 64, 32000
B, T = 64, 64
NCORES = 8
BL = B // NCORES            # batches per core
N = BL * T                  # tokens per core
P = 128                     # tokens per pair-chunk (2 batches)
NPAIR = N // P
SCALE = 1.0 / np.sqrt(E)
EPS = 1e-5
VC = 500                    # vocab cols per matmul
VG = 8                      # matmul chunks per staged DMA
NV = V // VC
NEG = -1.0e30
GROUPS = ((0, 1), (2, 3))

_PROG = None


def _build_program():
    import concourse.bass as bass
    import concourse.tile as tile
    from concourse import bacc, mybir
    from contextlib import ExitStack

    f32 = mybir.dt.float32
    bf16 = mybir.dt.bfloat16
    i32 = mybir.dt.int32
    A = mybir.ActivationFunctionType
    Op = mybir.AluOpType

    nc = bacc.Bacc("TRN2", target_bir_lowering=False, debug=False,
                   num_devices=NCORES)

    # ---- DRAM parameters ----
    d_idx = nc.dram_tensor("idx", [N], i32, kind="ExternalInput").ap()
    d_temb = nc.dram_tensor("temb", [V, E], f32, kind="ExternalInput").ap()
    d_wq = nc.dram_tensor("wq", [L, E + 1, E], bf16, kind="ExternalInput").ap()
    d_wk = nc.dram_tensor("wk", [L, E + 1, E], bf16, kind="ExternalInput").ap()
    d_wv = nc.dram_tensor("wv", [L, E + 1, 68], bf16, kind="ExternalInput").ap()
    d_wp = nc.dram_tensor("wp", [L, E + 1, E], bf16, kind="ExternalInput").ap()
    d_w1 = nc.dram_tensor("w1", [L, E + 1, 256], bf16, kind="ExternalInput").ap()
    d_w2 = nc.dram_tensor("w2", [L, 256, E], bf16, kind="ExternalInput").ap()
    d_b2 = nc.dram_tensor("b2", [L, 1, E], f32, kind="ExternalInput").ap()
    d_wlm = nc.dram_tensor("wlm", [E + 1, V], bf16, kind="ExternalInput").ap()
    d_pos = nc.dram_tensor("pos2", [P, E], f32, kind="ExternalInput").ap()
    d_mask = nc.dram_tensor("maskb", [P, 4 * T], f32, kind="ExternalInput").ap()
    d_ident = nc.dram_tensor("ident", [P, P], f32, kind="ExternalInput").ap()
    d_out = nc.dram_tensor("out", [N, V], f32, kind="ExternalOutput").ap()

    with tile.TileContext(nc) as tc:
        with ExitStack() as ctx:
            consts = ctx.enter_context(tc.tile_pool(name="consts", bufs=1))
            acts = ctx.enter_context(tc.tile_pool(name="acts", bufs=3))
            tmp = ctx.enter_context(tc.tile_pool(name="tmp", bufs=3))
            sbt = ctx.enter_context(tc.tile_pool(name="sbt", bufs=3))
            stg = ctx.enter_context(tc.tile_pool(name="stg", bufs=3))
            ps = ctx.enter_context(tc.tile_pool(name="ps", bufs=3, space="PSUM"))
            psu = ctx.enter_context(tc.tile_pool(name="psu", bufs=2, space="PSUM"))
            pslg = ctx.enter_context(tc.tile_pool(name="pslg", bufs=3, space="PSUM"))

            # ---- constants into SBUF ----
            wq_sb, wk_sb, wv_sb, wp_sb, w1_sb = [], [], [], [], []
            w2a_sb, w2b_sb, b2_sb = [], [], []
            for l in range(L):
                tq = consts.tile([E + 1, E], bf16, tag=f"wq{l}")
                nc.sync.dma_start(tq[:], d_wq[l])
                wq_sb.append(tq)
                tk = consts.tile([E + 1, E], bf16, tag=f"wk{l}")
                nc.sync.dma_start(tk[:], d_wk[l])
                wk_sb.append(tk)
                tv = consts.tile([E + 1, 68], bf16, tag=f"wv{l}")
                nc.sync.dma_start(tv[:], d_wv[l])
                wv_sb.append(tv)
                tp = consts.tile([E + 1, E], bf16, tag=f"wp{l}")
                nc.sync.dma_start(tp[:], d_wp[l])
                wp_sb.append(tp)
                t1 = consts.tile([E + 1, 256], bf16, tag=f"w1{l}")
                nc.sync.dma_start(t1[:], d_w1[l])
                w1_sb.append(t1)
                t2a = consts.tile([128, E], bf16, tag=f"w2a{l}")
                nc.sync.dma_start(t2a[:], d_w2[l, 0:128])
                w2a_sb.append(t2a)
                t2b = consts.tile([128, E], bf16, tag=f"w2b{l}")
                nc.sync.dma_start(t2b[:], d_w2[l, 128:256])
                w2b_sb.append(t2b)
                tb2 = consts.tile([P, E], f32, tag=f"b2{l}")
                b2bc = bass.AP(tensor=d_b2.tensor, offset=d_b2[l, 0].offset,
                               ap=[[0, P]] + [list(a) for a in d_b2[l, 0].ap])
                nc.gpsimd.dma_start(tb2[:], b2bc)
                b2_sb.append(tb2)
            wlm_sb = consts.tile([E + 1, V], bf16)
            nc.sync.dma_start(wlm_sb[:], d_wlm[:])
            pos_sb = consts.tile([P, E], f32)
            nc.sync.dma_start(pos_sb[:], d_pos[:])
            mask_sb = consts.tile([P, 4 * T], f32)
            nc.sync.dma_start(mask_sb[:], d_mask[:])
            ident_sb = consts.tile([P, P], f32)
            nc.sync.dma_start(ident_sb[:], d_ident[:])
            eps_sb = consts.tile([P, 1], f32)
            nc.vector.memset(eps_sb[:], EPS)

            from concourse.tile import add_dep_helper

            def layernorm(x, name, marks=None):
                """token-major LN: xhat [P, E] (affine folded into weights)."""
                st6 = tmp.tile([P, 6], f32, tag=f"st6_{name}")
                nc.vector.bn_stats(st6[:], x[:])
                mv = tmp.tile([P, 2], f32, tag=f"mv_{name}")
                nc.vector.bn_aggr(mv[:], st6[:])
                lnv = tmp.tile([P, 1], f32, tag=f"lnv_{name}")
                i_ln = nc.scalar.activation(lnv[:], mv[:, 1:2], A.Ln,
                                            bias=eps_sb[:])
                rstd = tmp.tile([P, 1], f32, tag=f"rstd_{name}")
                i_ex = nc.scalar.activation(rstd[:], lnv[:], A.Exp, scale=-0.5)
                xh = tmp.tile([P, E], f32, tag=f"xh_{name}")
                nc.vector.scalar_tensor_tensor(
                    out=xh[:], in0=x[:], scalar=mv[:, 0:1],
                    in1=rstd[:].to_broadcast([P, E]),
                    op0=Op.subtract, op1=Op.mult)
                if marks is not None:
                    marks.append((i_ln, i_ex))
                return xh

            def transpose_aug(src, name, copy_eng, dtype=f32):
                """[P, E]-tile -> [E+1, P] transposed + trailing ones row."""
                tps = ps.tile([E, P], f32, tag="ps")
                nc.tensor.transpose(tps[:], src[:], ident_sb[:])
                out = sbt.tile([E + 1, P], dtype, tag=f"tr_{name}")
                copy_eng(out[0:E, :], tps[:])
                nc.vector.memset(out[E:E + 1, :], 1.0)
                return out

            import concourse.bass as bass2

            def embed(p):
                idx_sb = tmp.tile([P, 1], i32, tag="idx")
                nc.sync.dma_start(idx_sb[:], d_idx[p * P:(p + 1) * P, None])
                xg = tmp.tile([P, E], f32, tag="xg")
                nc.gpsimd.indirect_dma_start(
                    out=xg[:], out_offset=None, in_=d_temb[:],
                    in_offset=bass2.IndirectOffsetOnAxis(ap=idx_sb[:, :1],
                                                         axis=0))
                x = acts.tile([P, E], f32, tag="x0")
                nc.vector.tensor_add(x[:], xg[:], pos_sb[:])
                return x

            def attention(x, l, entry_marks):
                xh = layernorm(x, "ln1", marks=entry_marks)
                hT = transpose_aug(xh, "h", nc.vector.tensor_copy, dtype=bf16)
                qps = ps.tile([D, H * P], f32, tag="ps")
                kps = ps.tile([D, H * P], f32, tag="ps")
                for h in range(H):
                    nc.tensor.matmul(qps[0:D, P * h:P * (h + 1)],
                                     lhsT=wq_sb[l][:, D * h:D * (h + 1)],
                                     rhs=hT[:])
                    nc.tensor.matmul(kps[0:D, P * h:P * (h + 1)],
                                     lhsT=wk_sb[l][:, D * h:D * (h + 1)],
                                     rhs=hT[:])
                qT = sbt.tile([D, H * P], f32, tag="qT")
                nc.scalar.copy(qT[:], qps[:])
                kT = sbt.tile([D, H * P], f32, tag="kT")
                nc.vector.tensor_copy(kT[:], kps[:])
                vps = ps.tile([P, 68], f32, tag="ps")
                nc.tensor.matmul(vps[:], lhsT=hT[:], rhs=wv_sb[l][:])
                v = sbt.tile([P, 68], f32, tag="v")
                nc.scalar.copy(v[:], vps[:])

                scps = ps.tile([P, 4 * T], f32, tag="ps")
                for h in range(H):
                    for b in range(2):
                        r0 = 64 * b
                        nc.tensor.matmul(
                            scps[r0:r0 + 64, 64 * h:64 * h + 64],
                            lhsT=kT[0:D, P * h + r0:P * h + r0 + 64],
                            rhs=qT[0:D, P * h + r0:P * h + r0 + 64])
                sm = sbt.tile([P, 4 * T], f32, tag="sm")
                nc.vector.scalar_tensor_tensor(
                    out=sm[:], in0=scps[:], scalar=float(SCALE),
                    in1=mask_sb[:], op0=Op.mult, op1=Op.add)
                ex = sbt.tile([P, 4 * T], f32, tag="ex")
                nc.scalar.activation(ex[:], sm[:], A.Exp)

                atps = ps.tile([P, 68], f32, tag="ps")
                for h in range(H):
                    for b in range(2):
                        r0 = 64 * b
                        nc.tensor.matmul(
                            atps[r0:r0 + 64, 17 * h:17 * h + 17],
                            lhsT=ex[r0:r0 + 64, 64 * h:64 * h + 64],
                            rhs=v[r0:r0 + 64, 17 * h:17 * h + 17])
                rr = tmp.tile([P, H], f32, tag="rr")
                nc.vector.reciprocal(rr[:], atps[:, 16::17])
                attn = tmp.tile([P, E], f32, tag="attn")
                at3 = atps[:].rearrange("p (h c) -> p h c", c=17)[:, :, 0:D]
                nc.vector.tensor_tensor(
                    out=attn[:].rearrange("p (h d) -> p h d", d=D),
                    in0=at3, in1=rr[:].to_broadcast([P, H, D]), op=Op.mult)

                aT = transpose_aug(attn, "a", nc.scalar.copy, dtype=bf16)
                pjps = ps.tile([P, E], f32, tag="ps")
                nc.tensor.matmul(pjps[:], lhsT=aT[:], rhs=wp_sb[l][:])
                x2 = acts.tile([P, E], f32, tag="x2")
                nc.vector.tensor_add(x2[:], x[:], pjps[:])
                return x2

            def mlp(x2, l, lastnl_marks, gelu_insts):
                xh2 = layernorm(x2, "ln2", marks=lastnl_marks)
                h2T = transpose_aug(xh2, "h2", nc.vector.tensor_copy,
                                    dtype=bf16)
                u1ps = psu.tile([P, P], f32, tag="psu")
                nc.tensor.matmul(u1ps[:], lhsT=w1_sb[l][:, 0:128], rhs=h2T[:])
                g1T = sbt.tile([P, P], bf16, tag="g1T")
                gelu_insts.append(nc.scalar.activation(g1T[:], u1ps[:], A.Gelu))
                u2ps = psu.tile([P, P], f32, tag="psu")
                nc.tensor.matmul(u2ps[:], lhsT=w1_sb[l][:, 128:256], rhs=h2T[:])
                g2T = sbt.tile([P, P], bf16, tag="g2T")
                gelu_insts.append(nc.scalar.activation(g2T[:], u2ps[:], A.Gelu))
                dps = ps.tile([P, E], f32, tag="ps")
                nc.tensor.matmul(dps[:], lhsT=g1T[:], rhs=w2a_sb[l][:],
                                 start=True, stop=False)
                nc.tensor.matmul(dps[:], lhsT=g2T[:], rhs=w2b_sb[l][:],
                                 start=False, stop=True)
                x3 = acts.tile([P, E], f32, tag="x3")
                nc.vector.tensor_add(x3[:], dps[:], b2_sb[l][:])
                x4 = acts.tile([P, E], f32, tag="x4")
                nc.vector.tensor_add(x4[:], x3[:], x2[:])
                return x4

            def lm_head(x, p, entry_marks):
                xf = layernorm(x, "lnf", marks=entry_marks)
                fT = transpose_aug(xf, "f", nc.vector.tensor_copy, dtype=bf16)
                for c2 in range(NV // VG):
                    stage = stg.tile([P, VG * VC], f32, tag="stage")
                    for j in range(VG):
                        c = VG * c2 + j
                        lg = pslg.tile([P, VC], f32, tag="lg")
                        nc.tensor.matmul(lg[:], lhsT=fT[:],
                                         rhs=wlm_sb[:, VC * c:VC * (c + 1)])
                        eng = (nc.vector.tensor_copy if j % 8 < 5
                               else nc.scalar.copy)
                        eng(stage[:, VC * j:VC * (j + 1)], lg[:])
                    nc.sync.dma_start(
                        d_out[p * P:(p + 1) * P,
                              VG * VC * c2:VG * VC * (c2 + 1)],
                        stage[:])

            # ACT table-set ordering: alternate clean "eras" of the
            # natural_log_exp set (LN/softmax ops) and the gelu set. Every
            # gelu cluster waits (ordering-only) on all pending nl-set exp
            # ops; every later Ln waits on the previous gelu cluster. This
            # keeps walrus's PSEUDO_LOAD_ACT_FUNC_SET count at ~2 per layer.
            prev_gelus = []
            pending_nl = []
            for grp in GROUPS:
                xs = {p: embed(p) for p in grp}
                for l in range(L):
                    entry, lastnl, gelus = [], [], []
                    for p in grp:
                        xs[p] = attention(xs[p], l, entry)
                    for p in grp:
                        xs[p] = mlp(xs[p], l, lastnl, gelus)
                    for i_ln, _ in entry:
                        for g in prev_gelus:
                            add_dep_helper(i_ln.ins, g.ins, sync=False,
                                           reason="act-set order")
                    for g in gelus:
                        for _, i_ex in pending_nl + lastnl:
                            add_dep_helper(g.ins, i_ex.ins, sync=False,
                                           reason="act-set order")
                    prev_gelus = gelus
                    pending_nl = []
                lnf_entry = []
                for p in grp:
                    lm_head(xs[p], p, lnf_entry)
                for i_ln, _ in lnf_entry:
                    for g in prev_gelus:
                        add_dep_helper(i_ln.ins, g.ins, sync=False,
                                       reason="act-set order")
                pending_nl = list(lnf_entry)

    nc.compile()
    return nc


def _prep_inputs(idx, tok_emb, pos_emb, Wq, Wk, Wv, Wproj, bproj,
                 ln1_g, ln1_b, ln2_g, ln2_b, W1, b1, W2, b2,
                 lnf_g, lnf_b, Wlm, blm):
    """Host-side weight folding/packing. Returns (shared inputs, per-core idx)."""
    import ml_dtypes
    f = np.float32
    bf = ml_dtypes.bfloat16
    idx = np.asarray(idx).astype(np.int32)
    tok_emb = np.asarray(tok_emb, f)
    pos_emb = np.asarray(pos_emb, f)
    Wq, Wk, Wv = np.asarray(Wq, f), np.asarray(Wk, f), np.asarray(Wv, f)
    Wproj, bproj = np.asarray(Wproj, f), np.asarray(bproj, f)
    ln1_g, ln1_b = np.asarray(ln1_g, f), np.asarray(ln1_b, f)
    ln2_g, ln2_b = np.asarray(ln2_g, f), np.asarray(ln2_b, f)
    W1, b1 = np.asarray(W1, f), np.asarray(b1, f)
    W2, b2 = np.asarray(W2, f), np.asarray(b2, f)
    lnf_g, lnf_b = np.asarray(lnf_g, f), np.asarray(lnf_b, f)
    Wlm, blm = np.asarray(Wlm, f), np.asarray(blm, f)

    wq_p = np.zeros((L, E + 1, E), f)
    wk_p = np.zeros((L, E + 1, E), f)
    wv_p = np.zeros((L, E + 1, 68), f)
    wp_p = np.zeros((L, E + 1, E), f)
    w1_p = np.zeros((L, E + 1, 256), f)
    w2_p = np.zeros((L, 256, E), f)
    b2_p = np.zeros((L, 1, E), f)
    for l in range(L):
        g1, b1l = ln1_g[l][:, None], ln1_b[l]
        for h in range(H):
            wq_p[l, :E, D * h:D * h + D] = g1 * Wq[l, h]
            wq_p[l, E, D * h:D * h + D] = b1l @ Wq[l, h]
            wk_p[l, :E, D * h:D * h + D] = g1 * Wk[l, h]
            wk_p[l, E, D * h:D * h + D] = b1l @ Wk[l, h]
            wv_p[l, :E, 17 * h:17 * h + D] = g1 * Wv[l, h]
            wv_p[l, E, 17 * h:17 * h + D] = b1l @ Wv[l, h]
            wv_p[l, E, 17 * h + D] = 1.0          # ones-column -> row sums
        wp_p[l, :E] = Wproj[l]
        wp_p[l, E] = bproj[l]
        w1_p[l, :E] = ln2_g[l][:, None] * W1[l]
        w1_p[l, E] = ln2_b[l] @ W1[l] + b1[l]
        w2_p[l] = W2[l]
        b2_p[l, 0] = b2[l]
    wlm_p = np.empty((E + 1, V), f)
    wlm_p[:E] = lnf_g[:, None] * Wlm
    wlm_p[E] = lnf_b @ Wlm + blm

    pos2 = np.concatenate([pos_emb, pos_emb], 0)          # [128, 64]
    m = np.where(np.arange(T)[:, None] <= np.arange(T)[None, :], 0, NEG)
    maskb = np.tile(np.concatenate([m, m], 0), (1, H)).astype(f)  # [128, 256]
    ident = np.eye(P, dtype=f)

    shared = dict(temb=tok_emb, wq=wq_p.astype(bf), wk=wk_p.astype(bf),
                  wv=wv_p.astype(bf), wp=wp_p.astype(bf), w1=w1_p.astype(bf),
                  w2=w2_p.astype(bf), b2=b2_p, wlm=wlm_p.astype(bf),
                  pos2=pos2, maskb=maskb, ident=ident)
    idx_cores = [idx[BL * i:BL * (i + 1)].reshape(N) for i in range(NCORES)]
    return shared, idx_cores


def make_in_maps(**inputs):
    shared, idx_cores = _prep_inputs(**inputs)
    return [dict(shared, idx=idx_cores[i]) for i in range(NCORES)]


def get_program():
    global _PROG
    if _PROG is None:
        _PROG = _build_program()
    return _PROG


def kernel(**inputs):
    from concourse.bass_utils import run_bass_kernel_spmd

    nc = get_program()
    in_maps = make_in_maps(**inputs)
    res = run_bass_kernel_spmd(nc, in_maps, list(range(NCORES)))
    outs = [res.results[i]["out"].reshape(BL, T, V) for i in range(NCORES)]
    return np.concatenate(outs, 0)


# revision 32
# speedup vs baseline: 1.1987x; 1.1310x over previous
"""NemoGPT (L=3, H=4, D=16, E=64, V=32000, B=64, T=64) on 8 Trainium2 cores.

Strategy: data-parallel over batch (8 batches/core = 512 tokens). Each core
runs the full transformer on its shard and streams its [512, 32000] f32
logits to DRAM. No collectives; the host concatenates per-core outputs.

Device structure (per core): 4 "pair" chunks of 128 tokens (= 2 batches),
processed in two groups of 2 pairs. Within a group the transformer runs
layer-major (minimizes ACT table-set switches between exp/ln and gelu);
each group's lm_head streaming overlaps the other group's compute.

Key layout tricks:
  - residual x token-major [128, E]; LN stats along the free dim; LN affine
    folded into the consuming weights (bias via a ones-row on the K dim)
  - scoresT [s, t] per head so exp(scoresT) is directly the attn lhsT
  - softmax without max-subtraction (scores ~ +-0.1); normalizer from a
    ones-column appended to V; all matmuls at PE row-group 0
  - MLP hidden computed transposed (uT = W1.T @ xhatT) so mlp2's lhsT is
    just gelu(uT) -- no transposes in the MLP
  - weight-side matmuls in bf16 (f32 PSUM accumulate); scores/attn in f32
  - lm_head: bf16 [65,128]x[65,500] -> f32 PSUM -> SBUF stage [128, 4000]
    -> 2 MiB DMAs
"""

import sys

for _p in ("/opt/trn_rl_repo", "/root/.axon_site", "/root/.axon_site/_ro/pypackages"):
    if _p not in sys.path:
        sys.path.insert(0, _p)

import numpy as np

L, H, D, E, V = 3, 4, 16, 64, 32000
B, T = 64, 64
NCORES = 8
BL = B // NCORES            # batches per core
N = BL * T                  # tokens per core
P = 128                     # tokens per pair-chunk (2 batches)
NPAIR = N // P
SCALE = 1.0 / np.sqrt(E)
EPS = 1e-5
VC = 500                    # vocab cols per matmul
VG = 8                      # matmul chunks per staged DMA
NV = V // VC
NEG = -1.0e30
GROUPS = ((0, 1), (2, 3))

_PROG = None


def _build_program():
    import concourse.bass as bass
    import concourse.tile as tile
    from concourse import bacc, mybir
    from contextlib import ExitStack

    f32 = mybir.dt.float32
    bf16 = mybir.dt.bfloat16
    i32 = mybir.dt.int32
    A = mybir.ActivationFunctionType
    Op = mybir.AluOpType

    # Steer bacc's activation-table-set assignment: by default it maps each
    # func to the first act_info.json set containing it (Exp ->
    # exp_and_others, Ln -> natural_log), which forces a ~1.3us table reload
    # between every Ln/Exp pair. Restricting Ln/Exp membership to
    # natural_log_exp_and_others keeps all LN + softmax ops in ONE set.
    # Set ids stay aligned with act_info.json (dict order is preserved).
    import functools
    from concourse import hw_specs as _hw

    if not getattr(bacc, "_act_tables_patched", False):
        _orig_gat = bacc.get_activation_tables

        @functools.cache
        def _patched_gat(arch):
            t = {k: set(v) for k, v in _orig_gat(arch).items()}
            if "natural_log_exp_and_others" in t:
                for k, fns in t.items():
                    if k != "natural_log_exp_and_others":
                        fns.discard(mybir.ActivationFunctionType.Exp)
                        fns.discard(mybir.ActivationFunctionType.Ln)
            return t

        bacc.get_activation_tables = _patched_gat
        bacc._act_tables_patched = True

    nc = bacc.Bacc("TRN2", target_bir_lowering=False, debug=False,
                   num_devices=NCORES)

    # ---- DRAM parameters ----
    d_idx = nc.dram_tensor("idx", [N], i32, kind="ExternalInput").ap()
    d_temb = nc.dram_tensor("temb", [V, E], f32, kind="ExternalInput").ap()
    d_wq = nc.dram_tensor("wq", [L, E + 1, E], bf16, kind="ExternalInput").ap()
    d_wk = nc.dram_tensor("wk", [L, E + 1, E], bf16, kind="ExternalInput").ap()
    d_wv = nc.dram_tensor("wv", [L, E + 1, 68], bf16, kind="ExternalInput").ap()
    d_wp = nc.dram_tensor("wp", [L, E + 1, E], bf16, kind="ExternalInput").ap()
    d_w1 = nc.dram_tensor("w1", [L, E + 1, 256], bf16, kind="ExternalInput").ap()
    d_w2 = nc.dram_tensor("w2", [L, 256, E], bf16, kind="ExternalInput").ap()
    d_b2 = nc.dram_tensor("b2", [L, 1, E], f32, kind="ExternalInput").ap()
    d_wlm = nc.dram_tensor("wlm", [E + 1, V], bf16, kind="ExternalInput").ap()
    d_pos = nc.dram_tensor("pos2", [P, E], f32, kind="ExternalInput").ap()
    d_mask = nc.dram_tensor("maskb", [P, 4 * T], f32, kind="ExternalInput").ap()
    d_ident = nc.dram_tensor("ident", [P, P], f32, kind="ExternalInput").ap()
    d_out = nc.dram_tensor("out", [N, V], f32, kind="ExternalOutput").ap()

    with tile.TileContext(nc) as tc:
        with ExitStack() as ctx:
            consts = ctx.enter_context(tc.tile_pool(name="consts", bufs=1))
            acts = ctx.enter_context(tc.tile_pool(name="acts", bufs=3))
            tmp = ctx.enter_context(tc.tile_pool(name="tmp", bufs=3))
            sbt = ctx.enter_context(tc.tile_pool(name="sbt", bufs=3))
            stg = ctx.enter_context(tc.tile_pool(name="stg", bufs=3))
            ps = ctx.enter_context(tc.tile_pool(name="ps", bufs=3, space="PSUM"))
            psu = ctx.enter_context(tc.tile_pool(name="psu", bufs=2, space="PSUM"))
            pslg = ctx.enter_context(tc.tile_pool(name="pslg", bufs=3, space="PSUM"))

            # ---- constants into SBUF ----
            wq_sb, wk_sb, wv_sb, wp_sb, w1_sb = [], [], [], [], []
            w2a_sb, w2b_sb, b2_sb = [], [], []
            for l in range(L):
                tq = consts.tile([E + 1, E], bf16, tag=f"wq{l}")
                nc.sync.dma_start(tq[:], d_wq[l])
                wq_sb.append(tq)
                tk = consts.tile([E + 1, E], bf16, tag=f"wk{l}")
                nc.sync.dma_start(tk[:], d_wk[l])
                wk_sb.append(tk)
                tv = consts.tile([E + 1, 68], bf16, tag=f"wv{l}")
                nc.sync.dma_start(tv[:], d_wv[l])
                wv_sb.append(tv)
                tp = consts.tile([E + 1, E], bf16, tag=f"wp{l}")
                nc.sync.dma_start(tp[:], d_wp[l])
                wp_sb.append(tp)
                t1 = consts.tile([E + 1, 256], bf16, tag=f"w1{l}")
                nc.sync.dma_start(t1[:], d_w1[l])
                w1_sb.append(t1)
                t2a = consts.tile([128, E], bf16, tag=f"w2a{l}")
                nc.sync.dma_start(t2a[:], d_w2[l, 0:128])
                w2a_sb.append(t2a)
                t2b = consts.tile([128, E], bf16, tag=f"w2b{l}")
                nc.sync.dma_start(t2b[:], d_w2[l, 128:256])
                w2b_sb.append(t2b)
                tb2 = consts.tile([P, E], f32, tag=f"b2{l}")
                b2bc = bass.AP(tensor=d_b2.tensor, offset=d_b2[l, 0].offset,
                               ap=[[0, P]] + [list(a) for a in d_b2[l, 0].ap])
                nc.gpsimd.dma_start(tb2[:], b2bc)
                b2_sb.append(tb2)
            wlm_sb = consts.tile([E + 1, V], bf16)
            nc.sync.dma_start(wlm_sb[:], d_wlm[:])
            pos_sb = consts.tile([P, E], f32)
            nc.sync.dma_start(pos_sb[:], d_pos[:])
            mask_sb = consts.tile([P, 4 * T], f32)
            nc.sync.dma_start(mask_sb[:], d_mask[:])
            ident_sb = consts.tile([P, P], f32)
            nc.sync.dma_start(ident_sb[:], d_ident[:])
            eps_sb = consts.tile([P, 1], f32)
            nc.vector.memset(eps_sb[:], EPS)

            from concourse.tile import add_dep_helper

            def layernorm(x, name, marks=None):
                """token-major LN: xhat [P, E] (affine folded into weights)."""
                st6 = tmp.tile([P, 6], f32, tag=f"st6_{name}")
                nc.vector.bn_stats(st6[:], x[:])
                mv = tmp.tile([P, 2], f32, tag=f"mv_{name}")
                nc.vector.bn_aggr(mv[:], st6[:])
                lnv = tmp.tile([P, 1], f32, tag=f"lnv_{name}")
                i_ln = nc.scalar.activation(lnv[:], mv[:, 1:2], A.Ln,
                                            bias=eps_sb[:])
                rstd = tmp.tile([P, 1], f32, tag=f"rstd_{name}")
                i_ex = nc.scalar.activation(rstd[:], lnv[:], A.Exp, scale=-0.5)
                xh = tmp.tile([P, E], f32, tag=f"xh_{name}")
                nc.vector.scalar_tensor_tensor(
                    out=xh[:], in0=x[:], scalar=mv[:, 0:1],
                    in1=rstd[:].to_broadcast([P, E]),
                    op0=Op.subtract, op1=Op.mult)
                if marks is not None:
                    marks.append((i_ln, i_ex))
                return xh

            def transpose_aug(src, name, copy_eng, dtype=f32):
                """[P, E]-tile -> [E+1, P] transposed + trailing ones row."""
                tps = ps.tile([E, P], f32, tag="ps")
                nc.tensor.transpose(tps[:], src[:], ident_sb[:])
                out = sbt.tile([E + 1, P], dtype, tag=f"tr_{name}")
                copy_eng(out[0:E, :], tps[:])
                nc.vector.memset(out[E:E + 1, :], 1.0)
                return out

            import concourse.bass as bass2

            def embed(p):
                idx_sb = tmp.tile([P, 1], i32, tag="idx")
                nc.sync.dma_start(idx_sb[:], d_idx[p * P:(p + 1) * P, None])
                xg = tmp.tile([P, E], f32, tag="xg")
                nc.gpsimd.indirect_dma_start(
                    out=xg[:], out_offset=None, in_=d_temb[:],
                    in_offset=bass2.IndirectOffsetOnAxis(ap=idx_sb[:, :1],
                                                         axis=0))
                x = acts.tile([P, E], f32, tag="x0")
                nc.vector.tensor_add(x[:], xg[:], pos_sb[:])
                return x

            def attention(x, l, entry_marks):
                xh = layernorm(x, "ln1", marks=entry_marks)
                hT = transpose_aug(xh, "h", nc.vector.tensor_copy, dtype=bf16)
                qps = ps.tile([D, H * P], f32, tag="ps")
                kps = ps.tile([D, H * P], f32, tag="ps")
                for h in range(H):
                    nc.tensor.matmul(qps[0:D, P * h:P * (h + 1)],
                                     lhsT=wq_sb[l][:, D * h:D * (h + 1)],
                                     rhs=hT[:])
                    nc.tensor.matmul(kps[0:D, P * h:P * (h + 1)],
                                     lhsT=wk_sb[l][:, D * h:D * (h + 1)],
                                     rhs=hT[:])
                qT = sbt.tile([D, H * P], f32, tag="qT")
                nc.scalar.copy(qT[:], qps[:])
                kT = sbt.tile([D, H * P], f32, tag="kT")
                nc.vector.tensor_copy(kT[:], kps[:])
                vps = ps.tile([P, 68], f32, tag="ps")
                nc.tensor.matmul(vps[:], lhsT=hT[:], rhs=wv_sb[l][:])
                v = sbt.tile([P, 68], f32, tag="v")
                nc.scalar.copy(v[:], vps[:])

                scps = ps.tile([P, 4 * T], f32, tag="ps")
                for h in range(H):
                    for b in range(2):
                        r0 = 64 * b
                        nc.tensor.matmul(
                            scps[r0:r0 + 64, 64 * h:64 * h + 64],
                            lhsT=kT[0:D, P * h + r0:P * h + r0 + 64],
                            rhs=qT[0:D, P * h + r0:P * h + r0 + 64])
                sm = sbt.tile([P, 4 * T], f32, tag="sm")
                nc.vector.scalar_tensor_tensor(
                    out=sm[:], in0=scps[:], scalar=float(SCALE),
                    in1=mask_sb[:], op0=Op.mult, op1=Op.add)
                ex = sbt.tile([P, 4 * T], f32, tag="ex")
                nc.scalar.activation(ex[:], sm[:], A.Exp)

                atps = ps.tile([P, 68], f32, tag="ps")
                for h in range(H):
                    for b in range(2):
                        r0 = 64 * b
                        nc.tensor.matmul(
                            atps[r0:r0 + 64, 17 * h:17 * h + 17],
                            lhsT=ex[r0:r0 + 64, 64 * h:64 * h + 64],
                            rhs=v[r0:r0 + 64, 17 * h:17 * h + 17])
                rr = tmp.tile([P, H], f32, tag="rr")
                nc.vector.reciprocal(rr[:], atps[:, 16::17])
                attn = tmp.tile([P, E], f32, tag="attn")
                at3 = atps[:].rearrange("p (h c) -> p h c", c=17)[:, :, 0:D]
                nc.vector.tensor_tensor(
                    out=attn[:].rearrange("p (h d) -> p h d", d=D),
                    in0=at3, in1=rr[:].to_broadcast([P, H, D]), op=Op.mult)

                aT = transpose_aug(attn, "a", nc.scalar.copy, dtype=bf16)
                pjps = ps.tile([P, E], f32, tag="ps")
                nc.tensor.matmul(pjps[:], lhsT=aT[:], rhs=wp_sb[l][:])
                x2 = acts.tile([P, E], f32, tag="x2")
                nc.vector.tensor_add(x2[:], x[:], pjps[:])
                return x2

            def mlp(x2, l, lastnl_marks, gelu_insts):
                xh2 = layernorm(x2, "ln2", marks=lastnl_marks)
                h2T = transpose_aug(xh2, "h2", nc.vector.tensor_copy,
                                    dtype=bf16)
                u1ps = psu.tile([P, P], f32, tag="psu")
                nc.tensor.matmul(u1ps[:], lhsT=w1_sb[l][:, 0:128], rhs=h2T[:])
                g1T = sbt.tile([P, P], bf16, tag="g1T")
                gelu_insts.append(nc.scalar.activation(g1T[:], u1ps[:], A.Gelu))
                u2ps = psu.tile([P, P], f32, tag="psu")
                nc.tensor.matmul(u2ps[:], lhsT=w1_sb[l][:, 128:256], rhs=h2T[:])
                g2T = sbt.tile([P, P], bf16, tag="g2T")
                gelu_insts.append(nc.scalar.activation(g2T[:], u2ps[:], A.Gelu))
                dps = ps.tile([P, E], f32, tag="ps")
                nc.tensor.matmul(dps[:], lhsT=g1T[:], rhs=w2a_sb[l][:],
                                 start=True, stop=False)
                nc.tensor.matmul(dps[:], lhsT=g2T[:], rhs=w2b_sb[l][:],
                                 start=False, stop=True)
                x3 = acts.tile([P, E], f32, tag="x3")
                nc.vector.tensor_add(x3[:], dps[:], b2_sb[l][:])
                x4 = acts.tile([P, E], f32, tag="x4")
                nc.vector.tensor_add(x4[:], x3[:], x2[:])
                return x4

            def lm_head(x, p, entry_marks):
                xf = layernorm(x, "lnf", marks=entry_marks)
                fT = transpose_aug(xf, "f", nc.vector.tensor_copy, dtype=bf16)
                for c2 in range(NV // VG):
                    stage = stg.tile([P, VG * VC], f32, tag="stage")
                    for j in range(VG):
                        c = VG * c2 + j
                        lg = pslg.tile([P, VC], f32, tag="lg")
                        nc.tensor.matmul(lg[:], lhsT=fT[:],
                                         rhs=wlm_sb[:, VC * c:VC * (c + 1)])
                        eng = (nc.vector.tensor_copy if j % 2 == 0
                               else nc.scalar.copy)
                        eng(stage[:, VC * j:VC * (j + 1)], lg[:])
                    nc.sync.dma_start(
                        d_out[p * P:(p + 1) * P,
                              VG * VC * c2:VG * VC * (c2 + 1)],
                        stage[:])

            # ACT table-set ordering: alternate clean "eras" of the
            # natural_log_exp set (LN/softmax ops) and the gelu set. Every
            # gelu cluster waits (ordering-only) on all pending nl-set exp
            # ops; every later Ln waits on the previous gelu cluster. This
            # keeps walrus's PSEUDO_LOAD_ACT_FUNC_SET count at ~2 per layer.
            prev_gelus = []
            pending_nl = []
            for grp in GROUPS:
                xs = {p: embed(p) for p in grp}
                for l in range(L):
                    entry, lastnl, gelus = [], [], []
                    for p in grp:
                        xs[p] = attention(xs[p], l, entry)
                    for p in grp:
                        xs[p] = mlp(xs[p], l, lastnl, gelus)
                    for i_ln, _ in entry:
                        for g in prev_gelus:
                            add_dep_helper(i_ln.ins, g.ins, sync=False,
                                           reason="act-set order")
                    for g in gelus:
                        for _, i_ex in pending_nl + lastnl:
                            add_dep_helper(g.ins, i_ex.ins, sync=False,
                                           reason="act-set order")
                    prev_gelus = gelus
                    pending_nl = []
                lnf_entry = []
                for p in grp:
                    lm_head(xs[p], p, lnf_entry)
                for i_ln, _ in lnf_entry:
                    for g in prev_gelus:
                        add_dep_helper(i_ln.ins, g.ins, sync=False,
                                       reason="act-set order")
                pending_nl = list(lnf_entry)

    nc.compile()
    return nc


def _prep_inputs(idx, tok_emb, pos_emb, Wq, Wk, Wv, Wproj, bproj,
                 ln1_g, ln1_b, ln2_g, ln2_b, W1, b1, W2, b2,
                 lnf_g, lnf_b, Wlm, blm):
    """Host-side weight folding/packing. Returns (shared inputs, per-core idx)."""
    import ml_dtypes
    f = np.float32
    bf = ml_dtypes.bfloat16
    idx = np.asarray(idx).astype(np.int32)
    tok_emb = np.asarray(tok_emb, f)
    pos_emb = np.asarray(pos_emb, f)
    Wq, Wk, Wv = np.asarray(Wq, f), np.asarray(Wk, f), np.asarray(Wv, f)
    Wproj, bproj = np.asarray(Wproj, f), np.asarray(bproj, f)
    ln1_g, ln1_b = np.asarray(ln1_g, f), np.asarray(ln1_b, f)
    ln2_g, ln2_b = np.asarray(ln2_g, f), np.asarray(ln2_b, f)
    W1, b1 = np.asarray(W1, f), np.asarray(b1, f)
    W2, b2 = np.asarray(W2, f), np.asarray(b2, f)
    lnf_g, lnf_b = np.asarray(lnf_g, f), np.asarray(lnf_b, f)
    Wlm, blm = np.asarray(Wlm, f), np.asarray(blm, f)

    wq_p = np.zeros((L, E + 1, E), f)
    wk_p = np.zeros((L, E + 1, E), f)
    wv_p = np.zeros((L, E + 1, 68), f)
    wp_p = np.zeros((L, E + 1, E), f)
    w1_p = np.zeros((L, E + 1, 256), f)
    w2_p = np.zeros((L, 256, E), f)
    b2_p = np.zeros((L, 1, E), f)
    for l in range(L):
        g1, b1l = ln1_g[l][:, None], ln1_b[l]
        for h in range(H):
            wq_p[l, :E, D * h:D * h + D] = g1 * Wq[l, h]
            wq_p[l, E, D * h:D * h + D] = b1l @ Wq[l, h]
            wk_p[l, :E, D * h:D * h + D] = g1 * Wk[l, h]
            wk_p[l, E, D * h:D * h + D] = b1l @ Wk[l, h]
            wv_p[l, :E, 17 * h:17 * h + D] = g1 * Wv[l, h]
            wv_p[l, E, 17 * h:17 * h + D] = b1l @ Wv[l, h]
            wv_p[l, E, 17 * h + D] = 1.0          # ones-column -> row sums
        wp_p[l, :E] = Wproj[l]
        wp_p[l, E] = bproj[l]
        w1_p[l, :E] = ln2_g[l][:, None] * W1[l]
        w1_p[l, E] = ln2_b[l] @ W1[l] + b1[l]
        w2_p[l] = W2[l]
        b2_p[l, 0] = b2[l]
    wlm_p = np.empty((E + 1, V), f)
    wlm_p[:E] = lnf_g[:, None] * Wlm
    wlm_p[E] = lnf_b @ Wlm + blm

    pos2 = np.concatenate([pos_emb, pos_emb], 0)          # [128, 64]
    m = np.where(np.arange(T)[:, None] <= np.arange(T)[None, :], 0, NEG)
    maskb = np.tile(np.concatenate([m, m], 0), (1, H)).astype(f)  # [128, 256]
    ident = np.eye(P, dtype=f)

    shared = dict(temb=tok_emb, wq=wq_p.astype(bf), wk=wk_p.astype(bf),
                  wv=wv_p.astype(bf), wp=wp_p.astype(bf), w1=w1_p.astype(bf),
                  w2=w2_p.astype(bf), b2=b2_p, wlm=wlm_p.astype(bf),
                  pos2=pos2, maskb=maskb, ident=ident)
    idx_cores = [idx[BL * i:BL * (i + 1)].reshape(N) for i in range(NCORES)]
    return shared, idx_cores


def make_in_maps(**inputs):
    shared, idx_cores = _prep_inputs(**inputs)
    return [dict(shared, idx=idx_cores[i]) for i in range(NCORES)]


def get_program():
    global _PROG
    if _PROG is None:
        _PROG = _build_program()
    return _PROG


def kernel(**inputs):
    from concourse.bass_utils import run_bass_kernel_spmd

    nc = get_program()
    in_maps = make_in_maps(**inputs)
    res = run_bass_kernel_spmd(nc, in_maps, list(range(NCORES)))
    outs = [res.results[i]["out"].reshape(BL, T, V) for i in range(NCORES)]
    return np.concatenate(outs, 0)
